# revision 39
# baseline (speedup 1.0000x reference)
"""Trainium2 Bass kernel for MultiHeadAttention (B=4, S=2048, D=1024, H=16, hd=64).

Sharding: 8 cores = batch(4) x head-group(2 groups of 8 heads).
Each core computes its batch's attention for its 8 heads plus the partial
output projection; the host sums the two partials per batch and adds bo.

Per-core device pipeline (all matmul inputs bf16, fp32 PSUM accumulation):
  1. V projection in [s, c] layout (x^T as stationary lhsT), bias via K=1
     ones-row matmul, per-head pad_k zeroing + scatter via tensor_scalar
     directly from PSUM into per-head [v|1|0] blocks.
  2. Q/K projections in [c, s] layout per head-pair (weights stationary).
     K-side bias is dropped (softmax over k is invariant to it); Q bias is
     added on DVE. Emission is ordered so work needing only the first half
     of x starts while the second half is still loading.
  3. Per (head, q-tile of 512): scores^T [k,q] matmuls (causal-skipped),
     exp via ACT (scale=1/8), diagonal-block triangular mask multiply,
     PV matmuls with stationary [v|ones|zeros] -> psum [c(64)+denom, q].
  4. Normalization: reciprocal_approx_fast on the PSUM denominator row
     (single fused DVE op), pad_q multiply on GpSimd, SBUF-source DMA
     broadcast to the 64 value partitions, multiply into values^T.
  5. Output projection out[s, dout] = values^T.T @ Wo_sub^T, DMA from PSUM
     via an SBUF copy.
"""

import numpy as np
import ml_dtypes

import concourse.bass as bass
import concourse.tile as tile
import concourse.mybir as mybir
from concourse import bacc
from concourse.bass_utils import run_bass_kernel_spmd

BF16 = mybir.dt.bfloat16
F32 = mybir.dt.float32
AF = mybir.ActivationFunctionType
ALU = mybir.AluOpType

B, S, D, H = 4, 2048, 1024, 16
HD = D // H            # 64
HL = H // 2            # 8 local heads per core
NP = HL // 2           # 4 head pairs per core
SC = S // 128          # 16 s-chunks
DC = D // 128          # 8 d-chunks
QT = S // 512          # 4 q-tiles
NB_K = S // 128        # 16 k-chunks

_NC_CACHE = {}


def build_kernel(causal=True):
    key = ("nc", causal)
    if key in _NC_CACHE:
        return _NC_CACHE[key]
    nc = bacc.Bacc("TRN2", target_bir_lowering=False)

    # ---- DRAM I/O (per core) ----
    xT_d = nc.dram_tensor("xT", [D, S], BF16, kind="ExternalInput")
    wq_d = nc.dram_tensor("wq", [D, HL * HD], BF16, kind="ExternalInput")
    wk_d = nc.dram_tensor("wk", [D, HL * HD], BF16, kind="ExternalInput")
    wv_d = nc.dram_tensor("wv", [D, HL * HD], BF16, kind="ExternalInput")
    wo_d = nc.dram_tensor("wo", [HL * HD, D], BF16, kind="ExternalInput")
    bq_d = nc.dram_tensor("bq", [NP, 128, 1], F32, kind="ExternalInput")
    bv_d = nc.dram_tensor("bv", [1, HL * HD], BF16, kind="ExternalInput")
    padk_d = nc.dram_tensor("padk", [SC, 128, 1], F32, kind="ExternalInput")
    padq_d = nc.dram_tensor("padq", [128, 8 * QT], F32, kind="ExternalInput")
    tri_d = nc.dram_tensor("tri", [128, 128], BF16, kind="ExternalInput")
    out_d = nc.dram_tensor("out", [S, D], F32, kind="ExternalOutput")

    with tile.TileContext(nc) as tc:
        with (
            tc.tile_pool(name="persist", bufs=1) as persist,
            tc.tile_pool(name="xpool", bufs=1) as xpool,
            tc.tile_pool(name="wpool", bufs=1) as wpool,
            tc.tile_pool(name="qk", bufs=1) as qkpool,
            tc.tile_pool(name="vals", bufs=1) as valpool,
            tc.tile_pool(name="probs", bufs=6) as probs_pool,
            tc.tile_pool(name="den", bufs=4) as den_pool,
            tc.tile_pool(name="wb", bufs=4) as wb_pool,
            tc.tile_pool(name="ost", bufs=4) as ost_pool,
            tc.tile_pool(name="dsc", bufs=4, space="DRAM") as dram_pool,
            tc.tile_pool(name="pspv", bufs=2, space="PSUM") as pspv,
            tc.tile_pool(name="psqk", bufs=2, space="PSUM") as psqk_pool,
            tc.tile_pool(name="ps2", bufs=2, space="PSUM") as ps2,
        ):
            # ---- persistent small tiles ----
            tri_sb = persist.tile([128, 128], BF16, tag="tri")
            nc.sync.dma_start(out=tri_sb[:], in_=tri_d[:, :])
            bq_sb = persist.tile([128, NP], F32, tag="bq")
            nc.sync.dma_start(out=bq_sb[:], in_=bq_d[:, :, :].rearrange("a p one -> p (a one)"))
            bv_sb = persist.tile([1, HL * HD], BF16, tag="bv")
            nc.sync.dma_start(out=bv_sb[:], in_=bv_d[:, :])
            padk_sb = persist.tile([128, SC], F32, tag="padk")
            nc.sync.dma_start(out=padk_sb[:], in_=padk_d[:, :, :].rearrange("c p one -> p (c one)"))
            padq_sb = persist.tile([128, 8 * QT], F32, tag="padq")
            nc.sync.dma_start(out=padq_sb[:], in_=padq_d[:, :])
            ones_sb = persist.tile([1, 128], BF16, tag="ones")
            nc.vector.memset(ones_sb[:], 1.0)

            # ---- v_sb static layout (zeros + ones cols), before any dep ----
            # v_sb[sc] layout [128, HL, 128]: head j even -> [v(64) | 1 | 0(63)],
            # head j odd  -> [0(63) | 1 | v(64)].
            # Even j: v at cols [0:64], ones col 64  -> psum rows v:[0:64], den:64
            # Odd  j: ones col 0, v at cols [64:128] -> psum rows den:0, v:[64:128]
            v_sb = [valpool.tile([128, HL, 128], BF16, tag=f"v{sc}", name=f"v{sc}") for sc in range(SC)]
            for sc in range(SC):
                nc.vector.memset(v_sb[sc][:], 0.0)
                for j in range(HL):
                    onecol = 64 if j % 2 == 0 else 0
                    nc.vector.memset(v_sb[sc][:, j, onecol : onecol + 1], 1.0)

            # ---- bulk loads: wv + x first half first, weights, x second half ----
            xT_sb = [xpool.tile([128, S], BF16, tag=f"xT{dc}", name=f"xT{dc}") for dc in range(DC)]
            wv_sb = [wpool.tile([128, HL * HD], BF16, tag=f"wv{dc}", name=f"wv{dc}") for dc in range(DC)]
            # sliver first: wv + x cols 0:256 unblock V sc0-1 ~8us earlier
            for dc in range(DC):
                nc.sync.dma_start(out=wv_sb[dc][:], in_=wv_d[bass.ts(dc, 128), :])
                nc.sync.dma_start(out=xT_sb[dc][:, 0:256], in_=xT_d[bass.ts(dc, 128), 0:256])
            for dc in range(DC):
                nc.sync.dma_start(out=xT_sb[dc][:, 256:1024], in_=xT_d[bass.ts(dc, 128), 256:1024])
            wq_sb = [wpool.tile([128, HL * HD], BF16, tag=f"wq{dc}", name=f"wq{dc}") for dc in range(DC)]
            wk_sb = [wpool.tile([128, HL * HD], BF16, tag=f"wk{dc}", name=f"wk{dc}") for dc in range(DC)]
            for dc in range(DC):
                nc.sync.dma_start(out=wq_sb[dc][:], in_=wq_d[bass.ts(dc, 128), :])
                nc.sync.dma_start(out=wk_sb[dc][:], in_=wk_d[bass.ts(dc, 128), :])
            for dc in range(DC):
                nc.sync.dma_start(out=xT_sb[dc][:, 1024:2048], in_=xT_d[bass.ts(dc, 128), 1024:2048])
            wo_sb = [wpool.tile([128, D], BF16, tag=f"wo{cc}", name=f"wo{cc}") for cc in range(4)]
            for cc in range(4):
                nc.sync.dma_start(out=wo_sb[cc][:], in_=wo_d[bass.ts(cc, 128), :])

            # ---- V projection: v[s, c] per s-chunk; lhsT = xT slice ----
            def emit_v(sc):
                psum_v2 = ps2.tile([128, 1024], F32, tag="ps2", name="psum_v2")
                psum_v = psum_v2[:, 0:512]
                for dc in range(DC):
                    nc.tensor.matmul(
                        psum_v[:],
                        lhsT=xT_sb[dc][:, bass.ts(sc, 128)],
                        rhs=wv_sb[dc][:],
                        start=(dc == 0),
                        stop=False,
                    )
                # bias via K=1 ones-row matmul
                nc.tensor.matmul(
                    psum_v[:],
                    lhsT=ones_sb[0:1, :],
                    rhs=bv_sb[0:1, :],
                    start=False,
                    stop=True,
                )
                # pad_k zeroing + cast + scatter: one batched tensor_scalar
                # per parity (even heads -> cols 0:64, odd heads -> 64:128)
                v4 = v_sb[sc][:].rearrange("p (e two) c -> p e two c", two=2)
                ps4 = psum_v[:].rearrange("p (e two c) -> p e two c", two=2, c=64)
                nc.vector.tensor_scalar_mul(
                    out=v4[:, :, 0:1, 0:64],
                    in0=ps4[:, :, 0:1, :],
                    scalar1=padk_sb[:, sc : sc + 1],
                )
                nc.vector.tensor_scalar_mul(
                    out=v4[:, :, 1:2, 64:128],
                    in0=ps4[:, :, 1:2, :],
                    scalar1=padk_sb[:, sc : sc + 1],
                )

            # ---- Q/K projections, staggered per pair ----
            qT_sb = [qkpool.tile([128, S], BF16, tag=f"qT{p}", name=f"qT{p}") for p in range(NP)]
            kT_sb = [qkpool.tile([128, S], BF16, tag=f"kT{p}", name=f"kT{p}") for p in range(NP)]

            def qk_gen(p):
                """Yield after each ~2-MM unit so QK projection work can be
                interleaved into the attention loop's exp-wait stalls."""
                for ss in range(QT):
                    psq = psqk_pool.tile([128, 512], F32, tag="psqk", name="psq")
                    for pair in range(4):
                        for dc in (2 * pair, 2 * pair + 1):
                            nc.tensor.matmul(
                                psq[:],
                                lhsT=wq_sb[dc][:, bass.ts(p, 128)],
                                rhs=xT_sb[dc][:, bass.ts(ss, 512)],
                                start=(dc == 0),
                                stop=(dc == DC - 1),
                            )
                        yield
                    nc.vector.tensor_scalar_add(
                        out=qT_sb[p][:, bass.ts(ss, 512)],
                        in0=psq[:],
                        scalar1=bq_sb[:, p : p + 1],
                    )
                    yield
                    psk = psqk_pool.tile([128, 512], F32, tag="psqk", name="psk")
                    for pair in range(4):
                        for dc in (2 * pair, 2 * pair + 1):
                            nc.tensor.matmul(
                                psk[:],
                                lhsT=wk_sb[dc][:, bass.ts(p, 128)],
                                rhs=xT_sb[dc][:, bass.ts(ss, 512)],
                                start=(dc == 0),
                                stop=(dc == DC - 1),
                            )
                        yield
                    nc.vector.tensor_copy(
                        out=kT_sb[p][:, bass.ts(ss, 512)],
                        in_=psk[:],
                    )
                    yield

            # half-0-only work first, then half-1 work (PE program order
            # matches data arrival so the array never stalls on half 1).
            for sc in range(8):
                emit_v(sc)
            for _ in qk_gen(0):
                pass
            for sc in range(8, 16):
                emit_v(sc)

            # ---- attention: per kc, both heads' score MMs issued adjacently
            # (row groups 0/64 run concurrently on HW) into the two banks of
            # one [128,1024] psum; ONE exp covers both heads; per-kc PV.
            valsT_sb = [valpool.tile([128, S], BF16, tag=f"valsT{cc}", name=f"valsT{cc}") for cc in range(NP)]
            # all denominators, [pp, p*32 + qt*8 + half*4 + i]
            den_all = persist.tile([128, NP * 32], F32, tag="den_all")

            def norm_finish(p):
                """Reciprocal + pad_q fold + broadcast + multiply for pair p's
                values; emitted AFTER p's attention so no hot-path op ever
                waits on the DMA bounce chain."""
                rcol = den_pool.tile([128, 32], F32, tag="rcol", name="rcol")
                nc.vector.reciprocal(out=rcol[:], in_=den_all[:, bass.ts(p, 32)])
                wcol = den_pool.tile([128, 32], BF16, tag="wcol", name="wcol")
                nc.vector.tensor_mul(out=wcol[:], in0=rcol[:], in1=padq_sb[:])
                for qt in range(QT):
                    for half in (0, 1):
                        hoff = half * 64
                        scr_b = dram_pool.tile([1, 512], BF16, tag=f"scrb{half}", name="scr_b")
                        nc.gpsimd.dma_start(
                            out=scr_b[0:1, :].rearrange("a (p f) -> (a p) f", p=128),
                            in_=wcol[:, qt * 8 + half * 4 : qt * 8 + (half + 1) * 4],
                        )
                        wb = wb_pool.tile([128, 512], BF16, tag="wb", name="wb")
                        nc.gpsimd.dma_start(
                            out=wb[hoff : hoff + 64, :],
                            in_=scr_b[0:1, :].to_broadcast([64, 512]),
                        )
                        nc.gpsimd.tensor_mul(
                            out=valsT_sb[p][hoff : hoff + 64, bass.ts(qt, 512)],
                            in0=valsT_sb[p][hoff : hoff + 64, bass.ts(qt, 512)],
                            in1=wb[hoff : hoff + 64, :],
                        )

            for p in range(NP):
                next_qk = qk_gen(p + 1) if p + 1 < NP else iter(())
                for qt in range(QT):
                    nkc = 4 * qt + 4 if causal else 16
                    ppv = {}
                    for half in (0, 1):
                        ppv[half] = pspv.tile([128, 512], F32, tag="pspv", name="ppv")
                    for kc in range(nkc):
                        if causal:
                            qs0 = max(qt * 512, kc * 128)
                        else:
                            qs0 = qt * 512
                        width = (qt + 1) * 512 - qs0
                        psc2 = ps2.tile([128, 1024], F32, tag="ps2", name="psc2")
                        for half in (0, 1):
                            hoff = half * 64
                            nc.tensor.matmul(
                                psc2[:, bass.ds(half * 512, width)],
                                lhsT=kT_sb[p][hoff : hoff + 64, bass.ts(kc, 128)],
                                rhs=qT_sb[p][hoff : hoff + 64, bass.ds(qs0, width)],
                                start=True,
                                stop=True,
                            )
                        pt = probs_pool.tile([128, 1024], BF16, tag="probs", name="pt")
                        if width == 512:
                            nc.scalar.activation(
                                out=pt[:], in_=psc2[:], func=AF.Exp, scale=0.125
                            )
                        else:
                            nc.scalar.activation(
                                out=pt[:].rearrange("a (h w) -> a h w", h=2)[:, :, :width],
                                in_=psc2[:].rearrange("a (h w) -> a h w", h=2)[:, :, :width],
                                func=AF.Exp,
                                scale=0.125,
                            )
                        if causal and kc >= 4 * qt:
                            nc.vector.tensor_mul(
                                out=pt[:, 0:128], in0=pt[:, 0:128], in1=tri_sb[:]
                            )
                            nc.vector.tensor_mul(
                                out=pt[:, 512:640], in0=pt[:, 512:640], in1=tri_sb[:]
                            )
                        for half in (0, 1):
                            j = 2 * p + half
                            nc.tensor.matmul(
                                ppv[half][:, bass.ds(qs0 - qt * 512, width)],
                                lhsT=v_sb[kc][:, j, :],
                                rhs=pt[:, bass.ds(half * 512, width)],
                                start=(kc == 0),
                                stop=(kc == nkc - 1),
                            )
                        # fill the exp-wait PE stall with one unit of the
                        # next head-pair's QK projection
                        next(next_qk, None)

                    # ---- denominator collection + PSUM drain only; the
                    # reciprocal/broadcast/multiply happen in norm_finish(p)
                    # after this pair's attention (nothing here blocks).
                    # sbrow row 0 = odd-head denom (half1), row 64 = even.
                    sbrow = den_pool.tile([128, 512], F32, tag="sbrow", name="sbrow")
                    for half in (0, 1):
                        dr = 64 if half == 0 else 0
                        hoff = half * 64
                        nc.vector.tensor_copy(
                            out=sbrow[dr : dr + 1, :],
                            in_=ppv[half][dr : dr + 1, :],
                        )
                        # unnormalized values off PSUM early (frees the PV
                        # accumulator for the next q-tile)
                        nc.vector.tensor_copy(
                            out=valsT_sb[p][hoff : hoff + 64, bass.ts(qt, 512)],
                            in_=ppv[half][hoff : hoff + 64, :],
                        )
                    for half in (0, 1):
                        dr = 64 if half == 0 else 0
                        scr_a = dram_pool.tile([1, 512], F32, tag=f"scra{half}", name="scr_a")
                        nc.sync.dma_start(out=scr_a[:], in_=sbrow[dr : dr + 1, :])
                        c0 = p * 32 + qt * 8 + half * 4
                        nc.sync.dma_start(
                            out=den_all[:, c0 : c0 + 4],
                            in_=scr_a[0:1, :].rearrange(
                                "a (p f) -> (a p) f", p=128
                            ),
                        )

                    if qt == 0 and p > 0:
                        norm_finish(p - 1)

                for _ in next_qk:
                    pass
            norm_finish(NP - 1)

            # ---- output projection ----
            for sc in range(SC):
                pso2 = ps2.tile([128, 1024], F32, tag="ps2", name="pso2")
                for do in range(2):
                    pso = pso2[:, bass.ds(do * 512, 512)]
                    for cc in range(NP):
                        nc.tensor.matmul(
                            pso[:],
                            lhsT=valsT_sb[cc][:, bass.ts(sc, 128)],
                            rhs=wo_sb[cc][:, bass.ds(do * 512, 512)],
                            start=(cc == 0),
                            stop=(cc == NP - 1),
                        )
                    ost = ost_pool.tile([128, 512], F32, tag="ost")
                    nc.vector.tensor_copy(out=ost[:], in_=pso[:])
                    nc.sync.dma_start(
                        out=out_d[bass.ts(sc, 128), bass.ds(do * 512, 512)],
                        in_=ost[:],
                    )

    nc.compile()
    _NC_CACHE[key] = nc
    return nc


def _prep_core_inputs(x, pad_mask, Wqkv, bqkv, Wo, b, hg):
    """Host-side shard prep for core (batch b, head-group hg)."""
    bf16 = ml_dtypes.bfloat16
    xT = np.ascontiguousarray(x[b].T).astype(bf16)  # [D, S]
    wq = np.empty((D, HL * HD), np.float32)
    wk = np.empty((D, HL * HD), np.float32)
    wv = np.empty((D, HL * HD), np.float32)
    bq = np.empty(HL * HD, np.float32)
    bv = np.empty(HL * HD, np.float32)
    for j in range(HL):
        gh = hg * HL + j
        r0 = gh * 3 * HD
        wq[:, j * HD : (j + 1) * HD] = Wqkv[r0 : r0 + HD, :].T
        wk[:, j * HD : (j + 1) * HD] = Wqkv[r0 + HD : r0 + 2 * HD, :].T
        wv[:, j * HD : (j + 1) * HD] = Wqkv[r0 + 2 * HD : r0 + 3 * HD, :].T
        bq[j * HD : (j + 1) * HD] = bqkv[r0 : r0 + HD]
        bv[j * HD : (j + 1) * HD] = bqkv[r0 + 2 * HD : r0 + 3 * HD]
    wo = np.ascontiguousarray(Wo[:, hg * HL * HD : (hg + 1) * HL * HD].T)  # [512, D]
    pad = pad_mask[b].astype(np.float32)  # [S]
    # padq in denominator-column layout: [pp, qt*8 + half*4 + i] =
    # pad[qt*512 + pp*4 + i], duplicated across the two halves.
    pq = pad.reshape(QT, 128, 4).transpose(1, 0, 2)  # [pp, qt, i]
    padq = np.ascontiguousarray(
        np.stack([pq, pq], axis=2).reshape(128, QT * 8)
    )
    tri = np.triu(np.ones((128, 128), np.float32))  # tri[k, q] = 1 if k <= q
    return {
        "xT": xT,
        "wq": wq.astype(bf16),
        "wk": wk.astype(bf16),
        "wv": wv.astype(bf16),
        "wo": wo.astype(bf16),
        "bq": bq.reshape(NP, 128, 1),
        "bv": bv.reshape(1, HL * HD).astype(bf16),
        "padk": pad.reshape(SC, 128, 1),
        "padq": padq,
        "tri": tri.astype(bf16),
    }


def run_sharded(inputs, trace=False):
    """Returns (full_output, BassKernelResults)."""
    x = np.asarray(inputs["x"], np.float32)
    pad_mask = np.asarray(inputs["pad_mask"])
    Wqkv = np.asarray(inputs["Wqkv"], np.float32)
    bqkv = np.asarray(inputs["bqkv"], np.float32)
    Wo = np.asarray(inputs["Wo"], np.float32)
    bo = np.asarray(inputs["bo"], np.float32)

    causal = bool(np.asarray(inputs.get("atn_mask", 1)).item())
    nc = build_kernel(causal=causal)
    in_maps = [
        _prep_core_inputs(x, pad_mask, Wqkv, bqkv, Wo, c // 2, c % 2)
        for c in range(8)
    ]
    res = run_bass_kernel_spmd(nc, in_maps, core_ids=list(range(8)), trace=trace)
    out = np.empty((B, S, D), np.float32)
    for b in range(B):
        out[b] = res.results[2 * b]["out"] + res.results[2 * b + 1]["out"] + bo
    return out, res


def kernel(**inputs):
    out, _ = run_sharded(inputs, trace=False)
    return out


# ---------------------------------------------------------------- benchmarking
def _build_sharded_exec(nc, n_cores=8):
    """Mirror bass2jax.run_bass_via_pjrt's multi-core path, reusable for
    repeated timed executions (keeps donation semantics)."""
    import jax
    import numpy as _np
    from jax.experimental.shard_map import shard_map
    from jax.sharding import Mesh, PartitionSpec, NamedSharding
    from concourse import bass2jax as b2j
    import concourse.mybir as _mybir

    b2j.install_neuronx_cc_hook()
    partition_name = nc.partition_id_tensor.name if nc.partition_id_tensor else None
    in_names, out_names, out_avals, zero_outs = [], [], [], []
    for alloc in nc.m.functions[0].allocations:
        if not isinstance(alloc, _mybir.MemoryLocationSet):
            continue
        name = alloc.memorylocations[0].name
        if alloc.kind == "ExternalInput":
            if name != partition_name:
                in_names.append(name)
        elif alloc.kind == "ExternalOutput":
            shape = tuple(alloc.tensor_shape)
            dtype = _mybir.dt.np(alloc.dtype)
            out_names.append(name)
            out_avals.append(jax.core.ShapedArray(shape, dtype))
            zero_outs.append(_np.zeros(shape, dtype))
    n_params = len(in_names)
    in_names = in_names + out_names
    donate = tuple(range(n_params, n_params + len(out_names)))

    def _body(*args):
        operands = list(args)
        if partition_name is not None:
            operands.append(b2j.partition_id_tensor())
        outs = b2j._bass_exec_p.bind(
            *operands,
            out_avals=tuple(out_avals),
            in_names=tuple(in_names),
            out_names=tuple(out_names),
            lowering_input_output_aliases=(),
            sim_require_finite=True,
            sim_require_nnan=True,
            nc=nc,
        )
        return tuple(outs)

    if partition_name is not None:
        in_names = in_names + [partition_name]
    devices = jax.devices()[:n_cores]
    mesh = Mesh(_np.asarray(devices), ("core",))
    spec = PartitionSpec("core")
    fn = jax.jit(
        shard_map(_body, mesh=mesh, in_specs=(spec,) * (n_params + len(out_names)),
                  out_specs=(spec,) * len(out_names), check_rep=False),
        donate_argnums=donate,
        keep_unused=True,
    )
    sharding = NamedSharding(mesh, spec)
    return fn, in_names[:n_params], out_names, zero_outs, sharding


def bench(inputs, iters=6):
    """Time repeated sharded executions. Returns (per_call_s list, outputs)."""
    import jax, time
    x = np.asarray(inputs["x"], np.float32)
    pad_mask = np.asarray(inputs["pad_mask"])
    Wqkv = np.asarray(inputs["Wqkv"], np.float32)
    bqkv = np.asarray(inputs["bqkv"], np.float32)
    Wo = np.asarray(inputs["Wo"], np.float32)

    nc = build_kernel()
    in_maps = [
        _prep_core_inputs(x, pad_mask, Wqkv, bqkv, Wo, c // 2, c % 2)
        for c in range(8)
    ]
    fn, in_names, out_names, zero_outs, sharding = _build_sharded_exec(nc)
    concat_in = [
        np.concatenate([np.asarray(in_maps[c][k]) for c in range(8)], axis=0)
        for k in in_names
    ]
    dev_in = [jax.device_put(a, sharding) for a in concat_in]
    zeros_proto = [np.zeros((8 * z.shape[0], *z.shape[1:]), z.dtype) for z in zero_outs]

    times = []
    out = None
    for it in range(iters + 1):
        dz = [jax.device_put(z, sharding) for z in zeros_proto]
        jax.block_until_ready(dz)
        t0 = time.perf_counter()
        out = fn(*dev_in, *dz)
        jax.block_until_ready(out)
        t1 = time.perf_counter()
        if it > 0:  # skip compile/warmup call
            times.append(t1 - t0)
    return times, out


def bench_chain(inputs, reps=(1, 33)):
    """Chain R kernel executions inside one jit dispatch (output of exec i
    feeds the donated out-buffer of exec i+1). Slope between rep counts gives
    per-exec device time without host/tunnel overhead."""
    import jax, time
    import numpy as _np
    from jax.experimental.shard_map import shard_map
    from jax.sharding import Mesh, PartitionSpec, NamedSharding
    from concourse import bass2jax as b2j
    import concourse.mybir as _mybir

    x = np.asarray(inputs["x"], np.float32)
    pad_mask = np.asarray(inputs["pad_mask"])
    Wqkv = np.asarray(inputs["Wqkv"], np.float32)
    bqkv = np.asarray(inputs["bqkv"], np.float32)
    Wo = np.asarray(inputs["Wo"], np.float32)
    nc = build_kernel()
    in_maps = [
        _prep_core_inputs(x, pad_mask, Wqkv, bqkv, Wo, c // 2, c % 2)
        for c in range(8)
    ]

    b2j.install_neuronx_cc_hook()
    partition_name = nc.partition_id_tensor.name if nc.partition_id_tensor else None
    in_names, out_names, out_avals, zero_outs = [], [], [], []
    for alloc in nc.m.functions[0].allocations:
        if not isinstance(alloc, _mybir.MemoryLocationSet):
            continue
        name = alloc.memorylocations[0].name
        if alloc.kind == "ExternalInput":
            if name != partition_name:
                in_names.append(name)
        elif alloc.kind == "ExternalOutput":
            shape = tuple(alloc.tensor_shape)
            dtype = _mybir.dt.np(alloc.dtype)
            out_names.append(name)
            out_avals.append(jax.core.ShapedArray(shape, dtype))
            zero_outs.append(_np.zeros(shape, dtype))
    n_params = len(in_names)
    all_names = in_names + out_names + ([partition_name] if partition_name else [])

    devices = jax.devices()[:8]
    mesh = Mesh(_np.asarray(devices), ("core",))
    spec = PartitionSpec("core")
    sharding = NamedSharding(mesh, spec)

    def make_fn(R):
        def _body(*args):
            params = list(args[:n_params])
            outs = list(args[n_params:])
            for _ in range(R):
                operands = params + outs
                if partition_name is not None:
                    operands.append(b2j.partition_id_tensor())
                outs = list(b2j._bass_exec_p.bind(
                    *operands,
                    out_avals=tuple(out_avals),
                    in_names=tuple(all_names),
                    out_names=tuple(out_names),
                    lowering_input_output_aliases=(),
                    sim_require_finite=True,
                    sim_require_nnan=True,
                    nc=nc,
                ))
            return tuple(outs)
        return jax.jit(
            shard_map(_body, mesh=mesh, in_specs=(spec,) * (n_params + len(out_names)),
                      out_specs=(spec,) * len(out_names), check_rep=False),
            keep_unused=True,
        )

    concat_in = [
        np.concatenate([np.asarray(in_maps[c][k]) for c in range(8)], axis=0)
        for k in in_names
    ]
    dev_in = [jax.device_put(a, sharding) for a in concat_in]
    dz = [jax.device_put(_np.zeros((8 * z.shape[0], *z.shape[1:]), z.dtype), sharding)
          for z in zero_outs]
    jax.block_until_ready(dev_in); jax.block_until_ready(dz)

    results = {}
    for R in reps:
        fn = make_fn(R)
        out = fn(*dev_in, *dz); jax.block_until_ready(out)  # compile+warm
        ts = []
        for _ in range(3):
            t0 = time.perf_counter()
            out = fn(*dev_in, *dz)
            jax.block_until_ready(out)
            ts.append(time.perf_counter() - t0)
        results[R] = min(ts)
    rs = sorted(results)
    if len(rs) >= 2:
        r0, r1 = rs[0], rs[-1]
        per_exec = (results[r1] - results[r0]) / (r1 - r0)
    else:
        per_exec = results[rs[0]]
    return per_exec, results
